# revision 1
# baseline (speedup 1.0000x reference)
"""AdaLayerNorm (ragged gather_csr + LayerNorm) Trainium2 Bass kernel.

Runs SPMD on 8 NeuronCores, data-parallel over the node dimension: each core
gets a contiguous 16384-row shard of `feat`, replicated affine weights, and
its segment end-offsets clipped to the local row range, so the gather_csr
expansion stays device-local (per the sharding hint).

Per-core pipeline (the kernel sits on a DVE/DMA ridge: ~9.2 us per 2 MB
chunk on the Vector engine vs ~7 us of DMA at ~420 GB/s over 16 engines):
- Chunked 2 MB loads in a p-major row layout: each of the 128 partitions
  holds 8 consecutive rows, so every input descriptor moves one contiguous
  16 KB span. Outputs are stored fp16 (|out| <= ~30 so the rounding is
  ~2e-4 relative; the host upcasts while unsharding), halving write traffic
  and keeping the 8 cores' combined demand under the chip HBM roofline.
- Engine balance: bn_stats + final multiply on DVE (the pacer); stats
  combine small-ops on GpSimd; segment mask + sqrt + normalize on ScalarE.
  (GPSIMD cannot touch PSUM and its is_lt is ~20x slower than DVE's, so the
  mask lives on ScalarE's Sign activation; fp16 compute measured ~25%
  SLOWER than fp32 on DVE/ACT, so the pipeline stays fp32 until the store.)
- The per-node segment broadcast g[seg(i), :] is computed on-device as ONE
  accumulating TensorE matmul per 128-row tile against a +-1 comparison
  matrix S01[s, i] = sign(end_s - 0.5 - row_i) (one ScalarE Sign activation
  per chunk; Sign shares an act table with Identity/Square/Sqrt so there is
  no table-switch cost) and telescoped rows dg[s] = g[s] - g[s+1] plus a
  65th always-on row holding g[0]:
      sum_s S01[s,i]*dg[s] + g[0] = 2*g[seg(row_i)]        (exact)
  The factor 2 is folded into alpha/beta via Sqrt(4*(var+eps)).
- dg comes straight out of the affine matmul: the host supplies
  adjacent-row differences of global_feat (fp16), so b cancels except on
  the last two rows (added via a two-hot vector) and the whole init chain
  is weights -> 5 fp16 matmuls -> one fp16 copy (~4 us), with in(0) first
  on the Sync ring and the fp16 weights right behind it.
- The normalize is IN-PLACE on the input tile; the multiply writes the
  fp16 output tile; per-engine iteration order is pinned with ordering
  edges (bn x8 -> mults -> recip on DVE; norms -> stats-tail -> sign on
  ACT) so the tile scheduler cannot defer applies and starve the output
  stream. skew-1 software pipeline; half-chunk output DMAs (quarters on
  the last two chunks to spread the final drain across DMA engines).
"""

import sys

sys.path.insert(0, "/opt/trn_rl_repo")

import os

import numpy as np

import concourse.bacc as bacc
import concourse.mybir as mybir
import concourse.tile as tile
from concourse.bass_utils import run_bass_kernel_spmd
from concourse.tile_rust import add_dep_helper


def _after(later, earlier, why):
    """Ordering-only edge: schedule `later` after `earlier` on its engine."""
    if later is not None and earlier is not None:
        add_dep_helper(later.ins, earlier.ins, sync=False, reason=why)

dt = mybir.dt
AF = mybir.ActivationFunctionType
ALU = mybir.AluOpType

EPS = 1e-5
P = 128
N_CORES = 8
N, D_FULL, S_FULL = 131072, 512, 64
ROWS = N // N_CORES


def _install_profshim():
    """Best-effort NTFF profiling hook for trace runs (optional)."""
    try:
        import types

        import antenv

        if getattr(antenv, "axon_hooks", None) is not None:
            return
        sys.path.insert(0, "/root/.axon_site/trn_agent_boot")
        import trn_boot

        hook = trn_boot._ntff_profile_via_ctypes("/opt/axon/libaxon_pjrt.so")
        m = types.ModuleType("antenv.axon_hooks")
        state = {"hook": hook}
        m.set_axon_ntff_profile_hook = lambda h: state.__setitem__("hook", h)
        m.get_axon_ntff_profile_hook = lambda: state["hook"]
        sys.modules["antenv.axon_hooks"] = m
        antenv.axon_hooks = m
    except Exception:
        pass


def build_kernel(
    rows=16384,
    D=512,
    S=64,
    chunk_tiles=8,
    skew=1,
    prefetch=6,
    bufs_in=8,
    bufs_sgn=5,
    bufs_sts=5,
):
    """Build the per-core Bass graph. All 8 cores run this same graph."""
    J = chunk_tiles
    chunk_rows = P * J
    nchunks = rows // chunk_rows
    assert rows % chunk_rows == 0
    WD = D
    KCH = WD // P

    S1 = S + 1
    nc = bacc.Bacc("TRN2", target_bir_lowering=False, debug=False)
    feat = nc.declare_dram_parameter("feat", [rows, D], dt.float32, isOutput=False)
    gfd = nc.declare_dram_parameter("gfd", [WD, S1], dt.float16, isOutput=False)
    WT = nc.declare_dram_parameter("WT", [WD, D], dt.float16, isOutput=False)
    bb = nc.declare_dram_parameter("b", [1, D], dt.float16, isOutput=False)
    ohi = nc.declare_dram_parameter("off_hi", [S1, nchunks], dt.float32, isOutput=False)
    # fp16 output store (|out| <= ~30 so fp16 rounding is ~2e-4 relative);
    # the host upcasts to fp32 while unsharding. Halves write traffic, which
    # keeps the 8 cores' combined demand under the chip HBM roofline.
    out = nc.declare_dram_parameter("out", [rows, D], dt.float16, isOutput=True)

    with tile.TileContext(nc) as tc:
        with (
            tc.tile_pool(name="const", bufs=1) as cst,
            tc.tile_pool(name="inb", bufs=bufs_in) as in_pool,
            tc.tile_pool(name="sgn", bufs=bufs_sgn) as sgn_pool,
            tc.tile_pool(name="o16", bufs=3) as o16_pool,
            tc.tile_pool(name="sts", bufs=bufs_sts) as sts_pool,
            tc.tile_pool(name="ps", bufs=2, space="PSUM") as ps_pool,
        ):
            feat_v = feat.ap().rearrange("(c p j) d -> c p j d", p=P, j=J)
            out_v = out.ap().rearrange("(c p j) d -> c p j d", p=P, j=J)
            ints = {}

            def dma_in(c, halves=False):
                ints[c] = in_pool.tile(
                    [P, J, D], dt.float32, tag="int", name=f"int{c}"
                )
                if halves:
                    # two logical DMAs: the first bn batch can start once the
                    # first half lands (cuts first-chunk latency)
                    H = J // 2
                    nc.sync.dma_start(ints[c][:, 0:H, :], feat_v[c][:, 0:H, :])
                    nc.sync.dma_start(ints[c][:, H:J, :], feat_v[c][:, H:J, :])
                else:
                    nc.sync.dma_start(ints[c][:], feat_v[c])

            # ------------- init: dg[s] = W @ (gf[s] - gf[s+1]) -------------
            # The host supplies already-differenced global features gfd
            # (rows 0..62: gf[s]-gf[s+1]; row 63: gf[63]; row 64: gf[0]), so
            # dg comes straight out of the affine matmul: b cancels in the
            # differences and is added only to rows 63/64 via a two-hot
            # vector. This collapses the init chain to weights -> 5 fp16
            # matmuls -> one fp16 copy (~4 us), unblocking the first apply.
            # (p k) interleave: partition p holds WD-rows 4p..4p+3, one
            # contiguous span per partition per DMA descriptor. Sync-ring
            # order: in(0) first, then weights, then the rest of the prefetch.
            dma_in(0, halves=True)
            wt_sb = cst.tile([P, KCH, D], dt.float16)
            nc.sync.dma_start(wt_sb[:], WT.ap().rearrange("(p k) d -> p k d", k=KCH))
            gfd_sb = cst.tile([P, KCH, S1], dt.float16)
            nc.sync.dma_start(gfd_sb[:], gfd.ap().rearrange("(p k) s -> p k s", k=KCH))
            # b/ohi are tiny but their ~66 small descriptors would sit between
            # the weights and in(1) on the Sync ring; the ACT queue has them
            # landed long before sign(0)/the b-matmul need them.
            b_sb = cst.tile([1, D], dt.float16)
            nc.scalar.dma_start(b_sb[:], bb.ap())
            ohi_sb = cst.tile([S1, nchunks], dt.float32)
            nc.scalar.dma_start(ohi_sb[:], ohi.ap())

            dma_in(1, halves=True)
            for c0 in range(2, min(prefetch, nchunks)):
                dma_in(c0)

            ehot = cst.tile([1, S1], dt.float16)
            nc.vector.memset(ehot[:], 0.0)
            nc.vector.memset(ehot[:, S - 1 : S1], 1.0)

            psg = ps_pool.tile([S1, D], dt.float32, tag="ps")
            for k in range(KCH):
                nc.tensor.matmul(
                    psg[:],
                    gfd_sb[:, k, :],
                    wt_sb[:, k, :],
                    start=(k == 0),
                    stop=False,
                )
            nc.tensor.matmul(psg[:], ehot[:], b_sb[:], start=False, stop=True)
            dg16 = cst.tile([S1, D], dt.float16)
            nc.scalar.activation(dg16[:], psg[:], AF.Copy)

            # p-major iota: flat column i*J + j holds row value j + J*i, so
            # sub-tile slice [:, j*P:(j+1)*P] column i maps to psum
            # partition i = local row i*J + j.
            iota = cst.tile([S1, chunk_rows], dt.float32)
            nc.gpsimd.iota(
                iota[:],
                pattern=[[1, J], [J, P]],
                base=0,
                channel_multiplier=0,
                allow_small_or_imprecise_dtypes=True,
            )

            # ---------------- main loop (software-pipelined) ----------------
            # Per-engine order is pinned with ordering edges; per iteration c
            # the engine queues see (skew=1):
            #   Sync: out_a(c-1), out_b(c-1), in(c+pf)
            #   DVE : bn x8(c), mult_a(c-1), mult_b(c-1)
            #   ACT : norm x8(c-1), msq(c), sqrt(c), sign(c)
            #   Pool: combine(c), alpha(c), beta(c)
            #   PE  : matmul x8(c-1)
            # The alpha/beta chain for chunk c completes by ~7 us into iter c
            # and is consumed at the start of iter c+1; sign(c) lands by the
            # iteration end and feeds PE's matmuls in iter c+1.
            def stats_a(c, prev_h):
                int_ = ints[c]
                st6 = sts_pool.tile([P, J, 6], dt.float32, tag="st6")
                bn_i = []
                for j in range(J):
                    bn_i.append(nc.vector.bn_stats(st6[:, j, :], int_[:, j, :]))
                _after(bn_i[0], prev_h.get("recip"), "DVE iter chain")
                return (int_, st6), {"bn_last": bn_i[-1]}

            def stats_b(c, staged, cur_h):
                int_, st6 = staged
                m_e = st6[:, :, 1]
                cv_e = st6[:, :, 2]
                m_o = st6[:, :, 4]
                cv_o = st6[:, :, 5]

                def emit_sign():
                    sh = sgn_pool.tile([S1, chunk_rows], dt.float16, tag="sh")
                    return sh, nc.scalar.activation(
                        sh[:],
                        iota[:],
                        AF.Sign,
                        bias=ohi_sb[:, c : c + 1],
                        scale=-1.0,
                    )

                msum = sts_pool.tile([P, J], dt.float32, tag="msum")
                nc.gpsimd.tensor_tensor(msum[:], m_e, m_o, ALU.add)
                mdif = sts_pool.tile([P, J], dt.float32, tag="mdif")
                nc.gpsimd.tensor_tensor(mdif[:], m_e, m_o, ALU.subtract)
                cvs = sts_pool.tile([P, J], dt.float32, tag="cvs")
                nc.gpsimd.tensor_tensor(cvs[:], cv_e, cv_o, ALU.add)
                negm = sts_pool.tile([P, J], dt.float32, tag="negm")
                nc.gpsimd.tensor_scalar(negm[:], msum[:], -0.5, None, op0=ALU.mult)
                msq = sts_pool.tile([P, J], dt.float32, tag="msq")
                msq_i = nc.scalar.activation(msq[:], mdif[:], AF.Square, scale=0.5)
                _after(msq_i, cur_h.get("norm_last"), "ACT stats-tail after norms")
                v = sts_pool.tile([P, J], dt.float32, tag="v")
                nc.gpsimd.tensor_scalar(
                    v[:], cvs[:], 1.0 / D, EPS, op0=ALU.mult, op1=ALU.add
                )
                nc.gpsimd.tensor_tensor(v[:], v[:], msq[:], ALU.add)
                # Sqrt(4v) = 2*sdev: folds the ps = 2*g factor into alpha/beta
                sdev = sts_pool.tile([P, J], dt.float32, tag="sdev")
                sqrt_i = nc.scalar.activation(sdev[:], v[:], AF.Sqrt, scale=4.0)
                alpha = sts_pool.tile([P, J], dt.float32, tag="alpha")
                recip_i = nc.vector.reciprocal(alpha[:], sdev[:])
                _after(recip_i, cur_h.get("mult_last"), "DVE recip after mults")
                cur_h["recip"] = recip_i
                beta = sts_pool.tile([P, J], dt.float32, tag="beta")
                nc.gpsimd.tensor_tensor(beta[:], negm[:], alpha[:], ALU.mult)
                # segment mask for chunk c, last on ACT this iteration
                sh, sign_i = emit_sign()
                _after(sign_i, sqrt_i, "ACT sign after stats tail")
                cur_h["sign"] = sign_i
                return (int_, sh, alpha, beta)

            def stats_b_head(c, staged, hb):
                # Last-iteration variant: everything up to Sqrt, emitted
                # before apply(c-1) so the reciprocal can interleave between
                # its two multiplies on DVE (beta would otherwise trail
                # mult_b by ~2us and gate the final chunk's norms).
                int_, st6 = staged
                m_e = st6[:, :, 1]
                cv_e = st6[:, :, 2]
                m_o = st6[:, :, 4]
                cv_o = st6[:, :, 5]
                msum = sts_pool.tile([P, J], dt.float32, tag="msum")
                nc.gpsimd.tensor_tensor(msum[:], m_e, m_o, ALU.add)
                mdif = sts_pool.tile([P, J], dt.float32, tag="mdif")
                nc.gpsimd.tensor_tensor(mdif[:], m_e, m_o, ALU.subtract)
                cvs = sts_pool.tile([P, J], dt.float32, tag="cvs")
                nc.gpsimd.tensor_tensor(cvs[:], cv_e, cv_o, ALU.add)
                negm = sts_pool.tile([P, J], dt.float32, tag="negm")
                nc.gpsimd.tensor_scalar(negm[:], msum[:], -0.5, None, op0=ALU.mult)
                msq = sts_pool.tile([P, J], dt.float32, tag="msq")
                hb["msq_i"] = nc.scalar.activation(msq[:], mdif[:], AF.Square, scale=0.5)
                v = sts_pool.tile([P, J], dt.float32, tag="v")
                nc.gpsimd.tensor_scalar(
                    v[:], cvs[:], 1.0 / D, EPS, op0=ALU.mult, op1=ALU.add
                )
                nc.gpsimd.tensor_tensor(v[:], v[:], msq[:], ALU.add)
                sdev = sts_pool.tile([P, J], dt.float32, tag="sdev")
                hb["sqrt_i"] = nc.scalar.activation(sdev[:], v[:], AF.Sqrt, scale=4.0)
                alpha = sts_pool.tile([P, J], dt.float32, tag="alpha")
                hb.update(int_=int_, negm=negm, sdev=sdev, alpha=alpha)

            def stats_b_tail(c, hb, cur_h):
                _after(hb["msq_i"], cur_h.get("norm_last"), "ACT stats-tail after norms")
                beta = sts_pool.tile([P, J], dt.float32, tag="beta")
                nc.gpsimd.tensor_tensor(beta[:], hb["negm"][:], hb["alpha"][:], ALU.mult)
                sh = sgn_pool.tile([S1, chunk_rows], dt.float16, tag="sh")
                sign_i = nc.scalar.activation(
                    sh[:],
                    iota[:],
                    AF.Sign,
                    bias=ohi_sb[:, c : c + 1],
                    scale=-1.0,
                )
                _after(sign_i, hb["sqrt_i"], "ACT sign after stats tail")
                cur_h["sign"] = sign_i
                return (hb["int_"], sh, hb["alpha"], beta)

            def apply_phase(c, staged, cur_h, prev_h, recip_hb=None):
                int_, sh, alpha, beta = staged
                ints.pop(c)
                o16 = o16_pool.tile([P, J, D], dt.float16, tag="o16")
                H = J // 2
                first_norm = True
                # The last chunk's out enqueues ride the (by then idle) ACT
                # queue right behind its own norms, in halves instead of
                # quarters, so the final drain isn't serialized behind
                # ~0.7us-per-enqueue Sync head-of-line waits.
                out_eng = nc.scalar if c == nchunks - 1 else nc.sync
                for grp in (slice(0, H), slice(H, J)):
                    # half-chunk PSUM tile from a 2-deep pool: PE can fill
                    # this group while DVE still drains the previous one,
                    # breaking the serialized PE->DVE handshake chain.
                    psh = ps_pool.tile([P, H, D], dt.float32, tag="ps")
                    for j in range(grp.start, grp.stop):
                        norm_i = nc.scalar.activation(
                            int_[:, j, :],
                            int_[:, j, :],
                            AF.Identity,
                            bias=beta[:, j : j + 1],
                            scale=alpha[:, j : j + 1],
                        )
                        if first_norm:
                            _after(norm_i, prev_h.get("sign"), "ACT norms after sign")
                            first_norm = False
                        nc.tensor.matmul(
                            psh[:, j - grp.start, :],
                            sh[:, j * P : (j + 1) * P],
                            dg16[:],
                            start=True,
                            stop=True,
                        )
                    mult_i = nc.vector.tensor_tensor(
                        o16[:, grp, :], int_[:, grp, :], psh[:], ALU.mult
                    )
                    if grp.start == 0:
                        _after(mult_i, cur_h.get("bn_last"), "DVE mults after bn batch")
                        if recip_hb is not None:
                            r_i = nc.vector.reciprocal(
                                recip_hb["alpha"][:], recip_hb["sdev"][:]
                            )
                            _after(r_i, mult_i, "mid recip after first mult")
                            recip_hb["recip_i"] = r_i
                    elif recip_hb is not None:
                        _after(mult_i, recip_hb["recip_i"], "second mult after recip")
                    cur_h["mult_last"] = mult_i
                    out_eng.dma_start(out_v[c][:, grp, :], o16[:, grp, :])
                cur_h["norm_last"] = norm_i

            staged = {}
            prev_h = {}
            for c in range(nchunks):
                part, cur_h = stats_a(c, prev_h)
                hb = None
                if c == nchunks - 1:
                    hb = {}
                    stats_b_head(c, part, hb)
                if c >= skew:
                    apply_phase(c - skew, staged.pop(c - skew), cur_h, prev_h, hb)
                if c + prefetch < nchunks:
                    dma_in(c + prefetch)
                if hb is None:
                    staged[c] = stats_b(c, part, cur_h)
                else:
                    staged[c] = stats_b_tail(c, hb, cur_h)
                prev_h = cur_h
            for c in range(nchunks - skew, nchunks):
                apply_phase(c, staged.pop(c), {}, prev_h)

    nc.compile()
    return nc


def make_in_maps(feat, global_feat, offset, W, b, n_cores=N_CORES, chunk_tiles=8):
    """Shard the full inputs into per-core in_maps."""
    N, D = feat.shape
    S = offset.shape[0]
    rows = N // n_cores
    chunk_rows = P * chunk_tiles
    nchunks = rows // chunk_rows
    feat = np.asarray(feat, dtype=np.float32)
    offset = np.asarray(offset, dtype=np.int64)
    # adjacent-row differences of global_feat (the telescoping terms); the
    # affine W-contraction itself runs on device.
    gfT32 = np.asarray(global_feat, dtype=np.float32).T
    gfd = np.empty((D, S + 1), dtype=np.float32)
    gfd[:, : S - 1] = gfT32[:, : S - 1] - gfT32[:, 1:S]
    gfd[:, S - 1] = gfT32[:, S - 1]
    gfd[:, S] = gfT32[:, 0]
    gfd = np.ascontiguousarray(gfd.astype(np.float16))
    WT = np.ascontiguousarray(np.asarray(W).T.astype(np.float16))
    b_ = np.asarray(b).reshape(1, D).astype(np.float16)
    ends = offset
    in_maps = []
    for c in range(n_cores):
        base = c * rows
        hi_c = np.clip(ends - base, 0, rows).astype(np.float32)
        # off_hi[s, ch] = hi_s - ch*chunk_rows - 0.5: Sign thresholds per
        # chunk; row S is a huge sentinel so its sign is always +1.
        off_hi = np.full((S + 1, nchunks), 1e9, dtype=np.float32)
        off_hi[:S, :] = (
            hi_c[:, None]
            - (np.arange(nchunks, dtype=np.float32) * chunk_rows)[None, :]
            - 0.5
        )
        in_maps.append(
            {
                "feat": np.ascontiguousarray(feat[base : base + rows]),
                "gfd": gfd,
                "WT": WT,
                "b": b_,
                "off_hi": np.ascontiguousarray(off_hi),
            }
        )
    return in_maps

_NC_CACHE = {}

last_exec_time_ns = None


def kernel(feat, global_feat, offset, W, b):
    """Full inputs in, full output out. Shards across 8 NeuronCores."""
    global last_exec_time_ns
    if "nc" not in _NC_CACHE:
        _NC_CACHE["nc"] = build_kernel(
            rows=ROWS,
            chunk_tiles=8,
            skew=1,
            prefetch=6,
            bufs_in=8,
        )
    nc = _NC_CACHE["nc"]
    in_maps = make_in_maps(feat, global_feat, offset, W, b, n_cores=N_CORES)
    kwargs = {}
    if os.environ.get("ADALN_TRACE") == "1":
        _install_profshim()
        import tempfile

        kwargs = {"trace": True, "tmpdir": tempfile.mkdtemp(prefix="adaln_prof_")}
    res = run_bass_kernel_spmd(nc, in_maps, core_ids=list(range(N_CORES)), **kwargs)
    last_exec_time_ns = res.exec_time_ns
    full = np.concatenate([res.results[i]["out"] for i in range(N_CORES)], axis=0)
    return full.astype(np.float32)



# revision 3
# speedup vs baseline: 1.4592x; 1.4592x over previous
"""AdaLayerNorm (ragged gather_csr + LayerNorm) Trainium2 Bass kernel.

Runs SPMD on 8 NeuronCores, data-parallel over the node dimension: each core
gets a contiguous 16384-row shard of `feat`, replicated affine weights, and
its segment end-offsets clipped to the local row range, so the gather_csr
expansion stays device-local (per the sharding hint).

Per-core pipeline (the kernel sits on a DVE/DMA ridge: ~9.2 us per 2 MB
chunk on the Vector engine vs ~7 us of DMA at ~420 GB/s over 16 engines):
- Chunked 2 MB loads in a p-major row layout: each of the 128 partitions
  holds 8 consecutive rows, so every input descriptor moves one contiguous
  16 KB span. Outputs are stored fp16 (|out| <= ~30 so the rounding is
  ~2e-4 relative; the host upcasts while unsharding), halving write traffic
  and keeping the 8 cores' combined demand under the chip HBM roofline.
- Engine balance: bn_stats + final multiply on DVE (the pacer); stats
  combine small-ops on GpSimd; segment mask + sqrt + normalize on ScalarE.
  (GPSIMD cannot touch PSUM and its is_lt is ~20x slower than DVE's, so the
  mask lives on ScalarE's Sign activation; fp16 compute measured ~25%
  SLOWER than fp32 on DVE/ACT, so the pipeline stays fp32 until the store.)
- The per-node segment broadcast g[seg(i), :] is computed on-device as ONE
  accumulating TensorE matmul per 128-row tile against a +-1 comparison
  matrix S01[s, i] = sign(end_s - 0.5 - row_i) (one ScalarE Sign activation
  per chunk; Sign shares an act table with Identity/Square/Sqrt so there is
  no table-switch cost) and telescoped rows dg[s] = g[s] - g[s+1] plus a
  65th always-on row holding g[0]:
      sum_s S01[s,i]*dg[s] + g[0] = 2*g[seg(row_i)]        (exact)
  The factor 2 is folded into alpha/beta via Sqrt(4*(var+eps)).
- dg comes straight out of the affine matmul: the host supplies
  adjacent-row differences of global_feat (fp16), so b cancels except on
  the last two rows (added via a two-hot vector) and the whole init chain
  is weights -> 5 fp16 matmuls -> one fp16 copy (~4 us), with in(0) first
  on the Sync ring and the fp16 weights right behind it.
- The normalize is IN-PLACE on the input tile; the multiply writes the
  fp16 output tile; per-engine iteration order is pinned with ordering
  edges (bn x8 -> mults -> recip on DVE; norms -> stats-tail -> sign on
  ACT) so the tile scheduler cannot defer applies and starve the output
  stream. skew-1 software pipeline; half-chunk output DMAs (quarters on
  the last two chunks to spread the final drain across DMA engines).
"""

import sys

sys.path.insert(0, "/opt/trn_rl_repo")

import os

import numpy as np

import concourse.bacc as bacc
import concourse.mybir as mybir
import concourse.tile as tile
from concourse.bass_utils import run_bass_kernel_spmd
from concourse.tile_rust import add_dep_helper


def _after(later, earlier, why):
    """Ordering-only edge: schedule `later` after `earlier` on its engine."""
    if later is not None and earlier is not None:
        add_dep_helper(later.ins, earlier.ins, sync=False, reason=why)

dt = mybir.dt
AF = mybir.ActivationFunctionType
ALU = mybir.AluOpType

EPS = 1e-5
P = 128
N_CORES = 8
N, D_FULL, S_FULL = 131072, 512, 64
ROWS = N // N_CORES


def _install_profshim():
    """Best-effort NTFF profiling hook for trace runs (optional)."""
    try:
        import types

        import antenv

        if getattr(antenv, "axon_hooks", None) is not None:
            return
        sys.path.insert(0, "/root/.axon_site/trn_agent_boot")
        import trn_boot

        hook = trn_boot._ntff_profile_via_ctypes("/opt/axon/libaxon_pjrt.so")
        m = types.ModuleType("antenv.axon_hooks")
        state = {"hook": hook}
        m.set_axon_ntff_profile_hook = lambda h: state.__setitem__("hook", h)
        m.get_axon_ntff_profile_hook = lambda: state["hook"]
        sys.modules["antenv.axon_hooks"] = m
        antenv.axon_hooks = m
    except Exception:
        pass


def build_kernel(
    rows=16384,
    D=512,
    S=64,
    chunk_tiles=8,
    skew=1,
    prefetch=6,
    bufs_in=8,
    bufs_sgn=5,
    bufs_sts=5,
):
    """Build the per-core Bass graph. All 8 cores run this same graph."""
    J = chunk_tiles
    chunk_rows = P * J
    nchunks = rows // chunk_rows
    assert rows % chunk_rows == 0
    WD = D
    KCH = WD // P

    S1 = S + 1
    nc = bacc.Bacc("TRN2", target_bir_lowering=False, debug=False)
    feat = nc.declare_dram_parameter("feat", [rows, D], dt.float32, isOutput=False)
    gfd = nc.declare_dram_parameter("gfd", [WD, S1], dt.float16, isOutput=False)
    WT = nc.declare_dram_parameter("WT", [WD, D], dt.float16, isOutput=False)
    bb = nc.declare_dram_parameter("b", [1, D], dt.float16, isOutput=False)
    ohi = nc.declare_dram_parameter("off_hi", [S1, nchunks], dt.float32, isOutput=False)
    # fp16 output store (|out| <= ~30 so fp16 rounding is ~2e-4 relative);
    # the host upcasts to fp32 while unsharding. Halves write traffic, which
    # keeps the 8 cores' combined demand under the chip HBM roofline.
    out = nc.declare_dram_parameter("out", [rows, D], dt.float16, isOutput=True)

    with tile.TileContext(nc) as tc:
        with (
            tc.tile_pool(name="const", bufs=1) as cst,
            tc.tile_pool(name="inb", bufs=bufs_in) as in_pool,
            tc.tile_pool(name="sgn", bufs=bufs_sgn) as sgn_pool,
            tc.tile_pool(name="o16", bufs=3) as o16_pool,
            tc.tile_pool(name="sts", bufs=bufs_sts) as sts_pool,
            tc.tile_pool(name="ps", bufs=2, space="PSUM") as ps_pool,
        ):
            feat_v = feat.ap().rearrange("(c p j) d -> c p j d", p=P, j=J)
            out_v = out.ap().rearrange("(c p j) d -> c p j d", p=P, j=J)
            ints = {}

            def dma_in(c, halves=False):
                ints[c] = in_pool.tile(
                    [P, J, D], dt.float32, tag="int", name=f"int{c}"
                )
                if halves:
                    # two logical DMAs: the first bn batch can start once the
                    # first half lands (cuts first-chunk latency)
                    H = J // 2
                    nc.sync.dma_start(ints[c][:, 0:H, :], feat_v[c][:, 0:H, :])
                    nc.sync.dma_start(ints[c][:, H:J, :], feat_v[c][:, H:J, :])
                else:
                    nc.sync.dma_start(ints[c][:], feat_v[c])

            # ------------- init: dg[s] = W @ (gf[s] - gf[s+1]) -------------
            # The host supplies already-differenced global features gfd
            # (rows 0..62: gf[s]-gf[s+1]; row 63: gf[63]; row 64: gf[0]), so
            # dg comes straight out of the affine matmul: b cancels in the
            # differences and is added only to rows 63/64 via a two-hot
            # vector. This collapses the init chain to weights -> 5 fp16
            # matmuls -> one fp16 copy (~4 us), unblocking the first apply.
            # (p k) interleave: partition p holds WD-rows 4p..4p+3, one
            # contiguous span per partition per DMA descriptor. Sync-ring
            # order: in(0) first, then weights, then the rest of the prefetch.
            # in(0) first quarter ahead of everything (earliest bn start),
            # weights right behind it, then the rest of in(0).
            ints[0] = in_pool.tile([P, J, D], dt.float32, tag="int", name="int0")
            nc.sync.dma_start(ints[0][:, 0:2, :], feat_v[0][:, 0:2, :])
            wt_sb = cst.tile([P, KCH, D], dt.float16)
            nc.sync.dma_start(wt_sb[:], WT.ap().rearrange("(p k) d -> p k d", k=KCH))
            gfd_sb = cst.tile([P, KCH, S1], dt.float16)
            nc.sync.dma_start(gfd_sb[:], gfd.ap().rearrange("(p k) s -> p k s", k=KCH))
            nc.sync.dma_start(ints[0][:, 2:4, :], feat_v[0][:, 2:4, :])
            nc.sync.dma_start(ints[0][:, 4:8, :], feat_v[0][:, 4:8, :])
            # b/ohi are tiny but their ~66 small descriptors would sit between
            # the weights and in(1) on the Sync ring; the ACT queue has them
            # landed long before sign(0)/the b-matmul need them.
            b_sb = cst.tile([1, D], dt.float16)
            nc.scalar.dma_start(b_sb[:], bb.ap())
            ohi_sb = cst.tile([S1, nchunks], dt.float32)
            nc.scalar.dma_start(ohi_sb[:], ohi.ap())

            dma_in(1, halves=True)
            for c0 in range(2, min(prefetch, nchunks)):
                dma_in(c0)

            ehot = cst.tile([1, S1], dt.float16)
            nc.vector.memset(ehot[:], 0.0)
            nc.vector.memset(ehot[:, S - 1 : S1], 1.0)

            psg = ps_pool.tile([S1, D], dt.float32, tag="ps")
            for k in range(KCH):
                nc.tensor.matmul(
                    psg[:],
                    gfd_sb[:, k, :],
                    wt_sb[:, k, :],
                    start=(k == 0),
                    stop=False,
                )
            nc.tensor.matmul(psg[:], ehot[:], b_sb[:], start=False, stop=True)
            dg16 = cst.tile([S1, D], dt.float16)
            nc.scalar.activation(dg16[:], psg[:], AF.Copy)

            # p-major iota: flat column i*J + j holds row value j + J*i, so
            # sub-tile slice [:, j*P:(j+1)*P] column i maps to psum
            # partition i = local row i*J + j.
            iota = cst.tile([S1, chunk_rows], dt.float32)
            nc.gpsimd.iota(
                iota[:],
                pattern=[[1, J], [J, P]],
                base=0,
                channel_multiplier=0,
                allow_small_or_imprecise_dtypes=True,
            )

            # ---------------- main loop (software-pipelined) ----------------
            # Per-engine order is pinned with ordering edges; per iteration c
            # the engine queues see (skew=1):
            #   Sync: out_a(c-1), out_b(c-1), in(c+pf)
            #   DVE : bn x8(c), mult_a(c-1), mult_b(c-1)
            #   ACT : norm x8(c-1), msq(c), sqrt(c), sign(c)
            #   Pool: combine(c), alpha(c), beta(c)
            #   PE  : matmul x8(c-1)
            # The alpha/beta chain for chunk c completes by ~7 us into iter c
            # and is consumed at the start of iter c+1; sign(c) lands by the
            # iteration end and feeds PE's matmuls in iter c+1.
            def stats_a(c, prev_h):
                int_ = ints[c]
                st6 = sts_pool.tile([P, J, 6], dt.float32, tag="st6")
                bn_i = []
                for j in range(J):
                    bn_i.append(nc.vector.bn_stats(st6[:, j, :], int_[:, j, :]))
                _after(bn_i[0], prev_h.get("recip"), "DVE iter chain")
                return (int_, st6), {"bn_last": bn_i[-1]}

            def stats_b(c, staged, cur_h):
                int_, st6 = staged
                m_e = st6[:, :, 1]
                cv_e = st6[:, :, 2]
                m_o = st6[:, :, 4]
                cv_o = st6[:, :, 5]

                def emit_sign():
                    sh = sgn_pool.tile([S1, chunk_rows], dt.float16, tag="sh")
                    return sh, nc.scalar.activation(
                        sh[:],
                        iota[:],
                        AF.Sign,
                        bias=ohi_sb[:, c : c + 1],
                        scale=-1.0,
                    )

                msum = sts_pool.tile([P, J], dt.float32, tag="msum")
                nc.gpsimd.tensor_tensor(msum[:], m_e, m_o, ALU.add)
                mdif = sts_pool.tile([P, J], dt.float32, tag="mdif")
                nc.gpsimd.tensor_tensor(mdif[:], m_e, m_o, ALU.subtract)
                cvs = sts_pool.tile([P, J], dt.float32, tag="cvs")
                nc.gpsimd.tensor_tensor(cvs[:], cv_e, cv_o, ALU.add)
                negm = sts_pool.tile([P, J], dt.float32, tag="negm")
                nc.gpsimd.tensor_scalar(negm[:], msum[:], -0.5, None, op0=ALU.mult)
                msq = sts_pool.tile([P, J], dt.float32, tag="msq")
                msq_i = nc.scalar.activation(msq[:], mdif[:], AF.Square, scale=0.5)
                _after(msq_i, cur_h.get("norm_last"), "ACT stats-tail after norms")
                v = sts_pool.tile([P, J], dt.float32, tag="v")
                nc.gpsimd.tensor_scalar(
                    v[:], cvs[:], 1.0 / D, EPS, op0=ALU.mult, op1=ALU.add
                )
                nc.gpsimd.tensor_tensor(v[:], v[:], msq[:], ALU.add)
                # Sqrt(4v) = 2*sdev: folds the ps = 2*g factor into alpha/beta
                sdev = sts_pool.tile([P, J], dt.float32, tag="sdev")
                sqrt_i = nc.scalar.activation(sdev[:], v[:], AF.Sqrt, scale=4.0)
                alpha = sts_pool.tile([P, J], dt.float32, tag="alpha")
                recip_i = nc.vector.reciprocal(alpha[:], sdev[:])
                _after(recip_i, cur_h.get("mult_last"), "DVE recip after mults")
                cur_h["recip"] = recip_i
                beta = sts_pool.tile([P, J], dt.float32, tag="beta")
                nc.gpsimd.tensor_tensor(beta[:], negm[:], alpha[:], ALU.mult)
                # segment mask for chunk c, last on ACT this iteration
                sh, sign_i = emit_sign()
                _after(sign_i, sqrt_i, "ACT sign after stats tail")
                cur_h["sign"] = sign_i
                return (int_, sh, alpha, beta)

            def stats_b_head(c, staged, hb):
                # Last-iteration variant: everything up to Sqrt, emitted
                # before apply(c-1) so the reciprocal can interleave between
                # its two multiplies on DVE (beta would otherwise trail
                # mult_b by ~2us and gate the final chunk's norms).
                int_, st6 = staged
                m_e = st6[:, :, 1]
                cv_e = st6[:, :, 2]
                m_o = st6[:, :, 4]
                cv_o = st6[:, :, 5]
                msum = sts_pool.tile([P, J], dt.float32, tag="msum")
                nc.gpsimd.tensor_tensor(msum[:], m_e, m_o, ALU.add)
                mdif = sts_pool.tile([P, J], dt.float32, tag="mdif")
                nc.gpsimd.tensor_tensor(mdif[:], m_e, m_o, ALU.subtract)
                cvs = sts_pool.tile([P, J], dt.float32, tag="cvs")
                nc.gpsimd.tensor_tensor(cvs[:], cv_e, cv_o, ALU.add)
                negm = sts_pool.tile([P, J], dt.float32, tag="negm")
                nc.gpsimd.tensor_scalar(negm[:], msum[:], -0.5, None, op0=ALU.mult)
                msq = sts_pool.tile([P, J], dt.float32, tag="msq")
                hb["msq_i"] = nc.scalar.activation(msq[:], mdif[:], AF.Square, scale=0.5)
                v = sts_pool.tile([P, J], dt.float32, tag="v")
                nc.gpsimd.tensor_scalar(
                    v[:], cvs[:], 1.0 / D, EPS, op0=ALU.mult, op1=ALU.add
                )
                nc.gpsimd.tensor_tensor(v[:], v[:], msq[:], ALU.add)
                sdev = sts_pool.tile([P, J], dt.float32, tag="sdev")
                hb["sqrt_i"] = nc.scalar.activation(sdev[:], v[:], AF.Sqrt, scale=4.0)
                alpha = sts_pool.tile([P, J], dt.float32, tag="alpha")
                hb.update(int_=int_, negm=negm, sdev=sdev, alpha=alpha)

            def stats_b_tail(c, hb, cur_h):
                _after(hb["msq_i"], cur_h.get("norm_last"), "ACT stats-tail after norms")
                beta = sts_pool.tile([P, J], dt.float32, tag="beta")
                nc.gpsimd.tensor_tensor(beta[:], hb["negm"][:], hb["alpha"][:], ALU.mult)
                sh = sgn_pool.tile([S1, chunk_rows], dt.float16, tag="sh")
                sign_i = nc.scalar.activation(
                    sh[:],
                    iota[:],
                    AF.Sign,
                    bias=ohi_sb[:, c : c + 1],
                    scale=-1.0,
                )
                _after(sign_i, hb["sqrt_i"], "ACT sign after stats tail")
                cur_h["sign"] = sign_i
                return (hb["int_"], sh, hb["alpha"], beta)

            def apply_phase(c, staged, cur_h, prev_h, recip_hb=None):
                int_, sh, alpha, beta = staged
                ints.pop(c)
                o16 = o16_pool.tile([P, J, D], dt.float16, tag="o16")
                H = J // 2
                first_norm = True
                # The last chunk's out enqueues ride the (by then idle) ACT
                # queue right behind its own norms, in halves instead of
                # quarters, so the final drain isn't serialized behind
                # ~0.7us-per-enqueue Sync head-of-line waits.
                out_eng = nc.scalar if c == nchunks - 1 else nc.sync
                for grp in (slice(0, H), slice(H, J)):
                    # half-chunk PSUM tile from a 2-deep pool: PE can fill
                    # this group while DVE still drains the previous one,
                    # breaking the serialized PE->DVE handshake chain.
                    psh = ps_pool.tile([P, H, D], dt.float32, tag="ps")
                    for j in range(grp.start, grp.stop):
                        norm_i = nc.scalar.activation(
                            int_[:, j, :],
                            int_[:, j, :],
                            AF.Identity,
                            bias=beta[:, j : j + 1],
                            scale=alpha[:, j : j + 1],
                        )
                        if first_norm:
                            _after(norm_i, prev_h.get("sign"), "ACT norms after sign")
                            first_norm = False
                        nc.tensor.matmul(
                            psh[:, j - grp.start, :],
                            sh[:, j * P : (j + 1) * P],
                            dg16[:],
                            start=True,
                            stop=True,
                        )
                    mult_i = nc.vector.tensor_tensor(
                        o16[:, grp, :], int_[:, grp, :], psh[:], ALU.mult
                    )
                    if grp.start == 0:
                        _after(mult_i, cur_h.get("bn_last"), "DVE mults after bn batch")
                        if recip_hb is not None:
                            r_i = nc.vector.reciprocal(
                                recip_hb["alpha"][:], recip_hb["sdev"][:]
                            )
                            _after(r_i, mult_i, "mid recip after first mult")
                            recip_hb["recip_i"] = r_i
                    elif recip_hb is not None:
                        _after(mult_i, recip_hb["recip_i"], "second mult after recip")
                    cur_h["mult_last"] = mult_i
                    if c == nchunks - 1:
                        # final drain: quarters alternating across the scalar
                        # and sync DMA queues so the tail transfers overlap
                        mid = (grp.start + grp.stop) // 2
                        nc.scalar.dma_start(
                            out_v[c][:, grp.start : mid, :],
                            o16[:, grp.start : mid, :],
                        )
                        nc.sync.dma_start(
                            out_v[c][:, mid : grp.stop, :],
                            o16[:, mid : grp.stop, :],
                        )
                    else:
                        out_eng.dma_start(out_v[c][:, grp, :], o16[:, grp, :])
                cur_h["norm_last"] = norm_i

            staged = {}
            prev_h = {}
            for c in range(nchunks):
                part, cur_h = stats_a(c, prev_h)
                hb = None
                if c == nchunks - 1:
                    hb = {}
                    stats_b_head(c, part, hb)
                if c >= skew:
                    apply_phase(c - skew, staged.pop(c - skew), cur_h, prev_h, hb)
                if c + prefetch < nchunks:
                    dma_in(c + prefetch)
                if hb is None:
                    staged[c] = stats_b(c, part, cur_h)
                else:
                    staged[c] = stats_b_tail(c, hb, cur_h)
                prev_h = cur_h
            for c in range(nchunks - skew, nchunks):
                apply_phase(c, staged.pop(c), {}, prev_h)

    nc.compile()
    return nc


def make_in_maps(feat, global_feat, offset, W, b, n_cores=N_CORES, chunk_tiles=8):
    """Shard the full inputs into per-core in_maps."""
    N, D = feat.shape
    S = offset.shape[0]
    rows = N // n_cores
    chunk_rows = P * chunk_tiles
    nchunks = rows // chunk_rows
    feat = np.asarray(feat, dtype=np.float32)
    offset = np.asarray(offset, dtype=np.int64)
    # adjacent-row differences of global_feat (the telescoping terms); the
    # affine W-contraction itself runs on device.
    gfT32 = np.asarray(global_feat, dtype=np.float32).T
    gfd = np.empty((D, S + 1), dtype=np.float32)
    gfd[:, : S - 1] = gfT32[:, : S - 1] - gfT32[:, 1:S]
    gfd[:, S - 1] = gfT32[:, S - 1]
    gfd[:, S] = gfT32[:, 0]
    gfd = np.ascontiguousarray(gfd.astype(np.float16))
    WT = np.ascontiguousarray(np.asarray(W).T.astype(np.float16))
    b_ = np.asarray(b).reshape(1, D).astype(np.float16)
    ends = offset
    in_maps = []
    for c in range(n_cores):
        base = c * rows
        hi_c = np.clip(ends - base, 0, rows).astype(np.float32)
        # off_hi[s, ch] = hi_s - ch*chunk_rows - 0.5: Sign thresholds per
        # chunk; row S is a huge sentinel so its sign is always +1.
        off_hi = np.full((S + 1, nchunks), 1e9, dtype=np.float32)
        off_hi[:S, :] = (
            hi_c[:, None]
            - (np.arange(nchunks, dtype=np.float32) * chunk_rows)[None, :]
            - 0.5
        )
        in_maps.append(
            {
                "feat": np.ascontiguousarray(feat[base : base + rows]),
                "gfd": gfd,
                "WT": WT,
                "b": b_,
                "off_hi": np.ascontiguousarray(off_hi),
            }
        )
    return in_maps

_NC_CACHE = {}

last_exec_time_ns = None


def kernel(feat, global_feat, offset, W, b):
    """Full inputs in, full output out. Shards across 8 NeuronCores."""
    global last_exec_time_ns
    if "nc" not in _NC_CACHE:
        _NC_CACHE["nc"] = build_kernel(
            rows=ROWS,
            chunk_tiles=8,
            skew=1,
            prefetch=6,
            bufs_in=8,
        )
    nc = _NC_CACHE["nc"]
    in_maps = make_in_maps(feat, global_feat, offset, W, b, n_cores=N_CORES)
    kwargs = {}
    if os.environ.get("ADALN_TRACE") == "1":
        _install_profshim()
        import tempfile

        kwargs = {"trace": True, "tmpdir": tempfile.mkdtemp(prefix="adaln_prof_")}
    res = run_bass_kernel_spmd(nc, in_maps, core_ids=list(range(N_CORES)), **kwargs)
    last_exec_time_ns = res.exec_time_ns
    full = np.concatenate([res.results[i]["out"] for i in range(N_CORES)], axis=0)
    return full.astype(np.float32)

